# revision 1
# baseline (speedup 1.0000x reference)
"""AverageAttention Trainium2 kernel.

Computes, per batch b (data-parallel across 8 NeuronCores):
    avg      = cumsum(x, axis=seq) / (pos+1)
    inter    = relu(LN(avg) @ w1 + b1)
    avg_out  = inter @ w2 + b2 + avg
    gates    = [x, avg_out] @ wg + bg
    gated    = sigmoid(gates[:, :D]) * x + sigmoid(gates[:, D:]) * avg_out
returns (gated, avg_out), each [B, S, D].

Implementation notes:
  - cumsum via triangular matmul per 128-seq block (fp32r: ~14-bit-mantissa
    fp32 streaming at full 1 cyc/row) + a K=1 rank-1 matmul that adds the
    running carry into PSUM, scaled by 1/(pos+1) at eviction (per-partition
    scale on ScalarE). The serial carry chain rides the ACT/GPSIMD queues so
    the busy DVE queue can't head-of-line-block it.
  - LayerNorm gain/bias are folded into w1/b1 on the host
    (w1' = ln_g[:,None]*w1, b1' = b1 + ln_b@w1), so on-chip LN is just
    (x-mu)*rstd via bn_stats/bn_aggr + one tensor_scalar.
  - FFN and gating matmuls run in bf16 (activations transposed on the PE
    with an identity matmul, cast at PSUM eviction); cumsum/LN stay fp32.
"""

import os
import sys

if "/opt/trn_rl_repo" not in sys.path:
    sys.path.insert(0, "/opt/trn_rl_repo")

# The NEFF executes via the axon-tunneled PJRT backend; a JAX_PLATFORMS=cpu
# pin (used for running references) would hide the NeuronCores.
if os.environ.get("JAX_PLATFORMS") == "cpu":
    os.environ.pop("JAX_PLATFORMS")

from contextlib import ExitStack

import ml_dtypes
import numpy as np

import concourse.bass as bass
import concourse.mybir as mybir
import concourse.tile as tile
from concourse import bacc
from concourse.bass_utils import run_bass_kernel_spmd

B, S, D = 8, 2048, 1024
P = 128
NBLK = S // P            # 16 seq blocks per core
CB = 2                   # seq blocks per pipeline chunk
NCHUNK = NBLK // CB
CS = CB * P              # chunk seq length (256)
D2 = 2 * D
KC = D // P              # 8 feature chunks of 128
EPS = 1e-6

FP32 = mybir.dt.float32
BF16 = mybir.dt.bfloat16
F32R = mybir.dt.float32r

AF = mybir.ActivationFunctionType
ALU = mybir.AluOpType


def build_program(has_b2: bool, has_bg: bool) -> bacc.Bacc:
    nc = bacc.Bacc("TRN2", target_bir_lowering=False, debug=False, num_devices=8)

    x_d = nc.declare_dram_parameter("x", [S, D], F32R, isOutput=False)
    w1_d = nc.declare_dram_parameter("w1g", [D, D], BF16, isOutput=False)
    b1_d = nc.declare_dram_parameter("b1p", [D], FP32, isOutput=False)
    w2_d = nc.declare_dram_parameter("w2", [D, D], BF16, isOutput=False)
    wg_d = nc.declare_dram_parameter("wg", [D2, D2], BF16, isOutput=False)
    tri_d = nc.declare_dram_parameter("tri", [P, P], F32R, isOutput=False)
    iden_d = nc.declare_dram_parameter("iden", [P, P], BF16, isOutput=False)
    inv_d = nc.declare_dram_parameter("invpos", [P, NBLK], FP32, isOutput=False)
    if has_b2:
        b2_d = nc.declare_dram_parameter("b2", [D], FP32, isOutput=False)
    if has_bg:
        bg_d = nc.declare_dram_parameter("bg", [D2], FP32, isOutput=False)

    gated_d = nc.declare_dram_parameter("gated", [S, D], FP32, isOutput=True)
    aout_d = nc.declare_dram_parameter("avg_out", [S, D], FP32, isOutput=True)

    x_r = x_d[:].rearrange("(n p) d -> p n d", p=P)        # [128, 16, 1024]
    aout_r = aout_d[:].rearrange("(n p) d -> p n d", p=P)
    gated_r = gated_d[:].rearrange("(n p) d -> p n d", p=P)
    w1_r = w1_d[:].rearrange("(c p) f -> p c f", p=P)      # [128, 8, 1024]
    w2_r = w2_d[:].rearrange("(c p) f -> p c f", p=P)
    wg_r = wg_d[:].rearrange("(c p) j -> p c j", p=P)      # [128, 16, 2048]

    with tile.TileContext(nc) as tc, ExitStack() as ctx:
        const = ctx.enter_context(tc.tile_pool(name="const", bufs=1))

        xT = const.tile([P, KC, S], BF16)      # x transposed, for gating lhsT
        aoT = const.tile([P, KC, S], BF16)     # avg_out transposed

        mm_ps = ctx.enter_context(tc.tile_pool(name="mm_ps", bufs=5, space="PSUM"))
        tot_ps = ctx.enter_context(tc.tile_pool(name="tot_ps", bufs=1, space="PSUM"))
        tr_ps = ctx.enter_context(tc.tile_pool(name="tr_ps", bufs=2, space="PSUM"))

        ctx1 = ctx.enter_context(ExitStack())
        w12 = ctx1.enter_context(tc.tile_pool(name="w12", bufs=1))
        xq_p = ctx1.enter_context(tc.tile_pool(name="xq", bufs=2))
        avgq_p = ctx1.enter_context(tc.tile_pool(name="avgq", bufs=2))
        zq_p = ctx1.enter_context(tc.tile_pool(name="zq", bufs=2))
        lnT_p = ctx1.enter_context(tc.tile_pool(name="lnT", bufs=2))
        intT_p = ctx1.enter_context(tc.tile_pool(name="intT", bufs=2))
        aoq_p = ctx1.enter_context(tc.tile_pool(name="aoq", bufs=2))
        cast_p = ctx1.enter_context(tc.tile_pool(name="cast", bufs=2))
        stat_p = ctx1.enter_context(tc.tile_pool(name="stat", bufs=6))
        incl_p = ctx1.enter_context(tc.tile_pool(name="incl", bufs=2))


        def transpose_blk(src_ap, dst_tile, dst_scol):
            """Transpose a [128, 1024] bf16 block into dst_tile[:, :, dst_scol:+128].

            8 PE transposes batched 4-per-PSUM-bank, evicted on ScalarE."""
            for h in range(2):
                ptr = tr_ps.tile([P, 512], BF16, tag="tr")
                for j in range(4):
                    k = 4 * h + j
                    nc.tensor.transpose(
                        ptr[:, j * P : (j + 1) * P],
                        src_ap[:, k * P : (k + 1) * P],
                        iden_sb,
                    )
                nc.scalar.copy(
                    out=dst_tile[:, 4 * h : 4 * h + 4, dst_scol : dst_scol + P],
                    in_=ptr[:].rearrange("p (j s) -> p j s", j=4),
                )

        x_tiles = {}

        def issue_x(qq):
            if qq >= NCHUNK:
                return
            t = xq_p.tile([P, CB, D], F32R)
            for bb in range(CB):
                nc.sync.dma_start(
                    out=t[:, bb, :], in_=x_r[:, qq * CB + bb, :]
                )
            x_tiles[qq] = t

        issue_x(0)
        issue_x(1)

        iden_sb = const.tile([P, P], BF16)
        nc.sync.dma_start(out=iden_sb, in_=iden_d[:])
        inv_sb = const.tile([P, NBLK], FP32)
        nc.sync.dma_start(out=inv_sb, in_=inv_d[:])
        b1t_sb = const.tile([P, KC], FP32)
        nc.sync.dma_start(out=b1t_sb, in_=b1_d[:].rearrange("(c p) -> p c", p=P))
        # int32 seed constant for the DVE fast-inverse-sqrt (keeps Sqrt off
        # ScalarE so the whole kernel fits one ACT table set — no mid-kernel
        # LoadActFuncSet switch before the gating sigmoids)
        magic_sb = const.tile([P, 1], mybir.dt.int32)
        nc.vector.memset(magic_sb, 0x5F3759DF)
        if has_b2:
            b2r_sb = const.tile([P, D], FP32)
            nc.sync.dma_start(out=b2r_sb, in_=b2_d[None, :].to_broadcast([P, D]))
        if has_bg:
            bgr_sb = const.tile([P, D2], FP32)
            nc.sync.dma_start(out=bgr_sb, in_=bg_d[None, :].to_broadcast([P, D2]))

        # fp32r operands may be DMA'd directly when the buffer dtype is f32r
        tri_rsb = const.tile([P, P], F32R)
        nc.sync.dma_start(out=tri_rsb, in_=tri_d[:])
        tri_r = tri_rsb[:]
        ones_row = tri_rsb[0:1, :]             # row of ones [1, 128]
        ones_col = tri_rsb[:, P - 1 : P]       # column of ones [128, 1]

        wg_pre = const.tile([P, KC, 512], BF16)  # wg[:, k<8, j 0:512] prefetch

        w1_sb = w12.tile([P, KC, D], BF16)
        nc.sync.dma_start(out=w1_sb, in_=w1_r)
        w2_sb = w12.tile([P, KC, D], BF16)
        nc.sync.dma_start(out=w2_sb, in_=w2_r)

        prev_incl = None
        for q in range(NCHUNK):
            x_q = x_tiles.pop(q)
            xr_q = x_q
            issue_x(q + 2)
            if q == NCHUNK - 2:
                nc.gpsimd.dma_start(out=wg_pre, in_=wg_r[:, :KC, 0:512])
            avg_q = avgq_p.tile([P, CB, D], FP32)
            z_q = zq_p.tile([P, CB, D], BF16)

            for b in range(CB):
                i = q * CB + b
                # -- in-block cumsum + carry, scaled to cumulative average ----
                # Row 127 of the raw in-block cumsum IS the block total; the
                # running prefix incl_i = incl_{i-1} + total_i rides on
                # ACT-copy + GPSIMD-add (rotating [1, D] tiles) so the serial
                # carry chain never touches the busy DVE queue.
                # -- running prefix of block sums (rotating [1, D] tiles);
                #    the serial carry chain rides ACT + GPSIMD queues --------
                if i < NBLK - 1:
                    cur_incl = incl_p.tile([1, D], F32R, tag="incl")
                    for c in range(2):
                        cs = slice(c * 512, (c + 1) * 512)
                        pst = tot_ps.tile([1, 512], FP32, tag="tot")
                        nc.tensor.matmul(
                            pst, lhsT=ones_col, rhs=xr_q[:, b, cs],
                            start=True, stop=True,
                        )
                        if i == 0:
                            nc.scalar.copy(out=cur_incl[0:1, cs], in_=pst)
                        else:
                            tot_sb = stat_p.tile([1, 512], FP32, tag="tot_sb")
                            nc.scalar.copy(out=tot_sb, in_=pst)
                            nc.gpsimd.tensor_add(
                                out=cur_incl[0:1, cs],
                                in0=prev_incl[0:1, cs],
                                in1=tot_sb,
                            )

                # -- in-block cumsum + carry, scaled to cumulative average ----
                for c in range(2):
                    cs = slice(c * 512, (c + 1) * 512)
                    ps = mm_ps.tile([P, 512], FP32, tag="mm")
                    nc.tensor.matmul(
                        ps, lhsT=tri_r, rhs=xr_q[:, b, cs],
                        start=True, stop=(i == 0),
                    )
                    if i > 0:
                        nc.tensor.matmul(
                            ps, lhsT=ones_row, rhs=prev_incl[0:1, cs],
                            start=False, stop=True,
                        )
                    nc.scalar.mul(out=avg_q[:, b, cs], in_=ps, mul=inv_sb[:, i : i + 1])
                if i < NBLK - 1:
                    prev_incl = cur_incl

                # -- LayerNorm stats + normalize (gain/bias folded into w1) ---
                st = stat_p.tile([P, 2, 6], FP32, tag="st")
                for g in range(2):
                    nc.vector.bn_stats(
                        out=st[:, g, :], in_=avg_q[:, b, g * 512 : (g + 1) * 512]
                    )
                mv = stat_p.tile([P, 2], FP32, tag="mv")
                nc.vector.bn_aggr(out=mv, in_=st)
                # rstd = 1/sqrt(var+eps) on DVE only: bit-hack seed + Newton
                y = stat_p.tile([P, 1], FP32, tag="y")
                nc.vector.tensor_scalar(
                    out=y, in0=mv[:, 1:2], scalar1=EPS, scalar2=None, op0=ALU.add
                )
                r0b = stat_p.tile([P, 1], mybir.dt.int32, tag="r0b")
                nc.vector.tensor_scalar(
                    out=r0b, in0=y[:].bitcast(mybir.dt.int32), scalar1=1,
                    scalar2=None, op0=ALU.logical_shift_right,
                )
                nc.vector.tensor_tensor(
                    out=r0b, in0=magic_sb, in1=r0b, op=ALU.subtract
                )
                rstd = r0b[:].bitcast(FP32)
                t = stat_p.tile([P, 1], FP32, tag="t")
                for _ in range(3):
                    nc.vector.tensor_tensor(out=t, in0=rstd, in1=rstd, op=ALU.mult)
                    nc.vector.tensor_tensor(out=t, in0=t, in1=y, op=ALU.mult)
                    nc.vector.tensor_scalar(
                        out=t, in0=t, scalar1=-0.5, scalar2=1.5,
                        op0=ALU.mult, op1=ALU.add,
                    )
                    nc.vector.tensor_tensor(out=rstd, in0=rstd, in1=t, op=ALU.mult)
                nc.vector.tensor_scalar(
                    out=z_q[:, b, :], in0=avg_q[:, b, :],
                    scalar1=mv[:, 0:1], scalar2=rstd,
                    op0=ALU.subtract, op1=ALU.mult,
                )
                if has_b2:
                    nc.gpsimd.tensor_add(
                        out=avg_q[:, b, :], in0=avg_q[:, b, :], in1=b2r_sb
                    )

                # -- transpose x block (bf16) for the gating matmul -----------
                xb = cast_p.tile([P, D], BF16, tag="xb")
                nc.vector.tensor_copy(out=xb, in_=x_q[:, b, :])
                transpose_blk(xb, xT, i * P)


            # -- transpose normalized activations: lnT [dpart, kc, cs] -------
            lnT_q = lnT_p.tile([P, KC, CS], BF16)
            for b in range(CB):
                transpose_blk(z_q[:, b, :], lnT_q, b * P)

            # -- FFN1: interT[f, s] = relu(w1'.T-chunks @ lnT + b1') ---------
            intT_q = intT_p.tile([P, KC, CS], BF16)
            for fc in range(KC):
                ps = mm_ps.tile([P, 512], FP32, tag="mm")
                for k in range(KC):
                    nc.tensor.matmul(
                        ps[:, :CS],
                        lhsT=w1_sb[:, k, fc * P : (fc + 1) * P],
                        rhs=lnT_q[:, k, :],
                        start=(k == 0), stop=(k == KC - 1),
                    )
                nc.scalar.activation(
                    out=intT_q[:, fc, :], in_=ps[:, :CS],
                    func=AF.Relu, bias=b1t_sb[:, fc : fc + 1],
                )

            # -- FFN2 + residual: avg_out = interT.T @ w2 + (avg + b2) -------
            ao_q = aoq_p.tile([P, CB, D], FP32)
            for b in range(CB):
                i = q * CB + b
                for dc in range(2):
                    ds_ = slice(dc * 512, (dc + 1) * 512)
                    ps = mm_ps.tile([P, 512], FP32, tag="mm")
                    for f in range(KC):
                        nc.tensor.matmul(
                            ps,
                            lhsT=intT_q[:, f, b * P : (b + 1) * P],
                            rhs=w2_sb[:, f, ds_],
                            start=(f == 0), stop=(f == KC - 1),
                        )
                    nc.vector.tensor_add(
                        out=ao_q[:, b, ds_], in0=ps, in1=avg_q[:, b, ds_]
                    )
                nc.sync.dma_start(out=aout_r[:, i, :], in_=ao_q[:, b, :])
                aob = cast_p.tile([P, D], BF16, tag="aob")
                nc.vector.tensor_copy(out=aob, in_=ao_q[:, b, :])
                transpose_blk(aob, aoT, i * P)

        # -- gating ------------------------------------------------------------
        ctx.callback(lambda: None)
        ctx1.close()
        wg_p = ctx.enter_context(tc.tile_pool(name="wg", bufs=2))
        sig_p = ctx.enter_context(tc.tile_pool(name="sig", bufs=4))
        re_p = ctx.enter_context(tc.tile_pool(name="re", bufs=3))
        g_p = ctx.enter_context(tc.tile_pool(name="g", bufs=3))

        for dh in range(2):  # output feature half (512 wide)
            ds_ = slice(dh * 512, (dh + 1) * 512)
            wg_in = wg_p.tile([P, 2 * KC, 512], BF16, tag="wgin")
            wg_fg = wg_p.tile([P, 2 * KC, 512], BF16, tag="wgfg")
            for kh in range(2):
                ks = slice(kh * KC, (kh + 1) * KC)
                if not (dh == 0 and kh == 0):
                    nc.gpsimd.dma_start(
                        out=wg_in[:, ks, :], in_=wg_r[:, ks, dh * 512 : (dh + 1) * 512]
                    )
                nc.gpsimd.dma_start(
                    out=wg_fg[:, ks, :],
                    in_=wg_r[:, ks, D + dh * 512 : D + (dh + 1) * 512],
                )
            for sb in range(NBLK):
                scol = slice(sb * P, (sb + 1) * P)
                ps_pair = []
                for wi, wgt in enumerate((wg_in, wg_fg)):
                    ps = mm_ps.tile([P, 512], FP32, tag="mm")
                    for k in range(2 * KC):
                        lhs = xT[:, k, scol] if k < KC else aoT[:, k - KC, scol]
                        if dh == 0 and wi == 0 and k < KC:
                            rhs = wg_pre[:, k, :]
                        else:
                            rhs = wgt[:, k, :]
                        nc.tensor.matmul(
                            ps, lhsT=lhs, rhs=rhs,
                            start=(k == 0), stop=(k == 2 * KC - 1),
                        )
                    ps_pair.append(ps)
                sig_in = sig_p.tile([P, 512], FP32, tag="sig")
                sig_fg = sig_p.tile([P, 512], FP32, tag="sig")
                if has_bg:
                    nc.vector.tensor_add(
                        out=sig_in, in0=ps_pair[0], in1=bgr_sb[:, ds_]
                    )
                    nc.scalar.activation(out=sig_in, in_=sig_in, func=AF.Sigmoid)
                    nc.vector.tensor_add(
                        out=sig_fg, in0=ps_pair[1],
                        in1=bgr_sb[:, D + dh * 512 : D + (dh + 1) * 512],
                    )
                    nc.scalar.activation(out=sig_fg, in_=sig_fg, func=AF.Sigmoid)
                else:
                    nc.scalar.activation(out=sig_in, in_=ps_pair[0], func=AF.Sigmoid)
                    nc.scalar.activation(out=sig_fg, in_=ps_pair[1], func=AF.Sigmoid)

                x_re = re_p.tile([P, 512], F32R, tag="xre")
                nc.sync.dma_start(out=x_re, in_=x_d[sb * P : (sb + 1) * P, ds_])
                ao_re = re_p.tile([P, 512], FP32, tag="aore")
                nc.sync.dma_start(out=ao_re, in_=aout_d[sb * P : (sb + 1) * P, ds_])
                m1 = g_p.tile([P, 512], FP32, tag="m1")
                nc.vector.tensor_mul(out=m1, in0=sig_in, in1=x_re)
                m2 = g_p.tile([P, 512], FP32, tag="m2")
                nc.gpsimd.tensor_mul(out=m2, in0=sig_fg, in1=ao_re)
                gt = g_p.tile([P, 512], FP32, tag="gt")
                nc.vector.tensor_add(out=gt, in0=m1, in1=m2)
                nc.sync.dma_start(out=gated_d[sb * P : (sb + 1) * P, ds_], in_=gt)

    nc.compile()
    return nc


def host_inputs(x, w1, b1, w2, b2, ln_g, ln_b, wg, bg):
    """Fold LN affine params into w1/b1, precompute constants, cast weights."""
    x = np.asarray(x, np.float32)
    w1 = np.asarray(w1, np.float32)
    w2 = np.asarray(w2, np.float32)
    wg = np.asarray(wg, np.float32)
    ln_g = np.asarray(ln_g, np.float32)
    ln_b = np.asarray(ln_b, np.float32)
    b1 = np.asarray(b1, np.float32)

    w1g = (ln_g[:, None] * w1).astype(ml_dtypes.bfloat16)
    b1p = (b1 + ln_b @ w1).astype(np.float32)
    tri = np.triu(np.ones((P, P), np.float32))
    iden = np.eye(P, dtype=ml_dtypes.bfloat16)
    pos = np.arange(S, dtype=np.float64).reshape(NBLK, P).T  # [P, NBLK]
    invpos = (1.0 / (pos + 1.0)).astype(np.float32)

    base = {
        "x": None,  # per-core
        "w1g": w1g,
        "b1p": b1p,
        "w2": w2.astype(ml_dtypes.bfloat16),
        "wg": wg.astype(ml_dtypes.bfloat16),
        "tri": tri,
        "iden": iden,
        "invpos": invpos,
    }
    has_b2 = bool(np.any(b2))
    has_bg = bool(np.any(bg))
    if has_b2:
        base["b2"] = np.asarray(b2, np.float32)
    if has_bg:
        base["bg"] = np.asarray(bg, np.float32)
    return base, has_b2, has_bg


_prog_cache = {}


def kernel(x, w1, b1, w2, b2, ln_g, ln_b, wg, bg):
    x = np.asarray(x, np.float32)
    assert x.shape == (B, S, D), x.shape
    base, has_b2, has_bg = host_inputs(x, w1, b1, w2, b2, ln_g, ln_b, wg, bg)

    key = (has_b2, has_bg)
    if key not in _prog_cache:
        _prog_cache[key] = build_program(has_b2, has_bg)
    nc = _prog_cache[key]

    in_maps = []
    for core in range(B):
        m = dict(base)
        m["x"] = np.ascontiguousarray(x[core])
        in_maps.append(m)

    res = run_bass_kernel_spmd(nc, in_maps, core_ids=list(range(B)))
    gated = np.stack([res.results[c]["gated"] for c in range(B)])
    avg_out = np.stack([res.results[c]["avg_out"] for c in range(B)])
    return gated, avg_out



# revision 9
# speedup vs baseline: 1.7782x; 1.7782x over previous
"""AverageAttention Trainium2 kernel.

Computes, per batch b (data-parallel across 8 NeuronCores):
    avg      = cumsum(x, axis=seq) / (pos+1)
    inter    = relu(LN(avg) @ w1 + b1)
    avg_out  = inter @ w2 + b2 + avg
    gates    = [x, avg_out] @ wg + bg
    gated    = sigmoid(gates[:, :D]) * x + sigmoid(gates[:, D:]) * avg_out
returns (gated, avg_out), each [B, S, D].

Implementation notes:
  - cumsum via triangular matmul per 128-seq block (fp32r: ~14-bit-mantissa
    fp32 streaming at full 1 cyc/row) + a K=1 rank-1 matmul that adds the
    running carry into PSUM, scaled by 1/(pos+1) at eviction (per-partition
    scale on ScalarE). The serial carry chain rides the ACT/GPSIMD queues so
    the busy DVE queue can't head-of-line-block it.
  - LayerNorm gain/bias are folded into w1/b1 on the host
    (w1' = ln_g[:,None]*w1, b1' = b1 + ln_b@w1), so on-chip LN is just
    (x-mu)*rstd via bn_stats/bn_aggr + one tensor_scalar.
  - FFN and gating matmuls run in bf16 (activations transposed on the PE
    with an identity matmul, cast at PSUM eviction); cumsum/LN stay fp32.
"""

import os
import sys

if "/opt/trn_rl_repo" not in sys.path:
    sys.path.insert(0, "/opt/trn_rl_repo")

# The NEFF executes via the axon-tunneled PJRT backend; a JAX_PLATFORMS=cpu
# pin (used for running references) would hide the NeuronCores.
if os.environ.get("JAX_PLATFORMS") == "cpu":
    os.environ.pop("JAX_PLATFORMS")

from contextlib import ExitStack

import ml_dtypes
import numpy as np

import concourse.bass as bass
import concourse.mybir as mybir
import concourse.tile as tile
from concourse import bacc
from concourse.bass_utils import run_bass_kernel_spmd

B, S, D = 8, 2048, 1024
P = 128
NBLK = S // P            # 16 seq blocks per core
CB = 2                   # seq blocks per pipeline chunk
NCHUNK = NBLK // CB
CS = CB * P              # chunk seq length (256)
D2 = 2 * D
KC = D // P              # 8 feature chunks of 128
EPS = 1e-6

FP32 = mybir.dt.float32
BF16 = mybir.dt.bfloat16
F32R = mybir.dt.float32r
FP8 = mybir.dt.float8e4
WS = 16.0                     # host-side weight scale (keeps fp8 normal-range)

AF = mybir.ActivationFunctionType
ALU = mybir.AluOpType
DR = mybir.MatmulPerfMode.DoubleRow


def build_program(has_b2: bool, has_bg: bool) -> bacc.Bacc:
    nc = bacc.Bacc("TRN2", target_bir_lowering=False, debug=False, num_devices=8)

    x_d = nc.declare_dram_parameter("x", [S, D], F32R, isOutput=False)
    w1_d = nc.declare_dram_parameter("w1g", [D, D], FP8, isOutput=False)
    b1_d = nc.declare_dram_parameter("b1p", [D], FP32, isOutput=False)
    w2_d = nc.declare_dram_parameter("w2", [D, D], FP8, isOutput=False)
    wg_d = nc.declare_dram_parameter("wg", [D2, D2], FP8, isOutput=False)
    tri_d = nc.declare_dram_parameter("tri", [P, P], F32R, isOutput=False)
    iden_d = nc.declare_dram_parameter("iden", [P, P], BF16, isOutput=False)
    inv_d = nc.declare_dram_parameter("invpos", [P, NBLK], FP32, isOutput=False)
    if has_b2:
        b2_d = nc.declare_dram_parameter("b2", [D], FP32, isOutput=False)
    if has_bg:
        bg_d = nc.declare_dram_parameter("bg", [D2], FP32, isOutput=False)

    gated_d = nc.declare_dram_parameter("gated", [S, D], FP32, isOutput=True)
    aout_d = nc.declare_dram_parameter("avg_out", [S, D], FP32, isOutput=True)

    x_r = x_d[:].rearrange("(n p) d -> p n d", p=P)        # [128, 16, 1024]
    aout_r = aout_d[:].rearrange("(n p) d -> p n d", p=P)
    gated_r = gated_d[:].rearrange("(n p) d -> p n d", p=P)
    w1_r = w1_d[:].rearrange("(c p) f -> p c f", p=P)      # [128, 8, 1024]
    w2_r = w2_d[:].rearrange("(c p) f -> p c f", p=P)
    wg_r = wg_d[:].rearrange("(c p) j -> p c j", p=P)      # [128, 16, 2048]

    with tile.TileContext(nc) as tc, ExitStack() as ctx:
        const = ctx.enter_context(tc.tile_pool(name="const", bufs=1))

        xT = const.tile([P, KC, S], FP8)       # x transposed, for gating lhsT
        aoT = const.tile([P, KC, S], FP8)      # avg_out transposed

        mm_ps = ctx.enter_context(tc.tile_pool(name="mm_ps", bufs=5, space="PSUM"))
        tot_ps = ctx.enter_context(tc.tile_pool(name="tot_ps", bufs=1, space="PSUM"))
        tr_ps = ctx.enter_context(tc.tile_pool(name="tr_ps", bufs=2, space="PSUM"))

        ctx1 = ctx.enter_context(ExitStack())
        w12 = ctx1.enter_context(tc.tile_pool(name="w12", bufs=1))
        xq_p = ctx1.enter_context(tc.tile_pool(name="xq", bufs=2))
        avgq_p = ctx1.enter_context(tc.tile_pool(name="avgq", bufs=2))
        zq_p = ctx1.enter_context(tc.tile_pool(name="zq", bufs=2))
        lnT_p = ctx1.enter_context(tc.tile_pool(name="lnT", bufs=2))
        intT_p = ctx1.enter_context(tc.tile_pool(name="intT", bufs=2))
        aoq_p = ctx1.enter_context(tc.tile_pool(name="aoq", bufs=2))
        cast_p = ctx1.enter_context(tc.tile_pool(name="cast", bufs=2))
        stat_p = ctx1.enter_context(tc.tile_pool(name="stat", bufs=6))
        incl_p = ctx1.enter_context(tc.tile_pool(name="incl", bufs=2))


        def transpose_blk(src_ap, dst_tile, dst_scol):
            """Transpose a [128, 1024] bf16 block into dst_tile[:, :, dst_scol:+128].

            8 PE transposes batched 4-per-PSUM-bank, evicted on ScalarE."""
            for h in range(2):
                ptr = tr_ps.tile([P, 512], BF16, tag="tr")
                for j in range(4):
                    k = 4 * h + j
                    nc.tensor.transpose(
                        ptr[:, j * P : (j + 1) * P],
                        src_ap[:, k * P : (k + 1) * P],
                        iden_sb,
                    )
                nc.scalar.copy(
                    out=dst_tile[:, 4 * h : 4 * h + 4, dst_scol : dst_scol + P],
                    in_=ptr[:].rearrange("p (j s) -> p j s", j=4),
                )

        x_tiles = {}

        def issue_x(qq):
            if qq >= NCHUNK:
                return
            t = xq_p.tile([P, CB, D], F32R)
            for bb in range(CB):
                nc.sync.dma_start(
                    out=t[:, bb, :], in_=x_r[:, qq * CB + bb, :]
                )
            x_tiles[qq] = t

        issue_x(0)
        issue_x(1)

        iden_sb = const.tile([P, P], BF16)
        nc.sync.dma_start(out=iden_sb, in_=iden_d[:])
        inv_sb = const.tile([P, NBLK], FP32)
        nc.sync.dma_start(out=inv_sb, in_=inv_d[:])
        b1t_sb = const.tile([P, KC], FP32)
        nc.sync.dma_start(out=b1t_sb, in_=b1_d[:].rearrange("(c p) -> p c", p=P))
        # int32 seed constant for the DVE fast-inverse-sqrt (keeps Sqrt off
        # ScalarE so the whole kernel fits one ACT table set — no mid-kernel
        # LoadActFuncSet switch before the gating sigmoids)
        magic_sb = const.tile([P, 1], mybir.dt.int32)
        nc.vector.memset(magic_sb, 0x5F3759DF)
        if has_b2:
            b2r_sb = const.tile([P, D], FP32)
            nc.sync.dma_start(out=b2r_sb, in_=b2_d[None, :].to_broadcast([P, D]))
        if has_bg:
            bgr_sb = const.tile([P, D2], FP32)
            nc.sync.dma_start(out=bgr_sb, in_=bg_d[None, :].to_broadcast([P, D2]))

        # fp32r operands may be DMA'd directly when the buffer dtype is f32r
        tri_rsb = const.tile([P, P], F32R)
        nc.sync.dma_start(out=tri_rsb, in_=tri_d[:])
        tri_r = tri_rsb[:]
        ones_row = tri_rsb[0:1, :]             # row of ones [1, 128]
        ones_col = tri_rsb[:, P - 1 : P]       # column of ones [128, 1]

        wg_pre = const.tile([P, KC, 512], FP8)  # wg[:, k<8, j 0:512] prefetch

        w1_sb = w12.tile([P, KC, D], FP8)
        nc.sync.dma_start(out=w1_sb, in_=w1_r)
        w2_sb = w12.tile([P, KC, D], FP8)
        nc.sync.dma_start(out=w2_sb, in_=w2_r)

        prev_incl = None
        for q in range(NCHUNK):
            x_q = x_tiles.pop(q)
            xr_q = x_q
            issue_x(q + 2)
            if q == NCHUNK - 2:
                nc.gpsimd.dma_start(out=wg_pre, in_=wg_r[:, :KC, 0:512])
            avg_q = avgq_p.tile([P, CB, D], FP32)
            z_q = zq_p.tile([P, CB, D], BF16)

            for b in range(CB):
                i = q * CB + b
                # -- in-block cumsum + carry, scaled to cumulative average ----
                # Row 127 of the raw in-block cumsum IS the block total; the
                # running prefix incl_i = incl_{i-1} + total_i rides on
                # ACT-copy + GPSIMD-add (rotating [1, D] tiles) so the serial
                # carry chain never touches the busy DVE queue.
                # -- running prefix of block sums (rotating [1, D] tiles);
                #    the serial carry chain rides ACT + GPSIMD queues --------
                if i < NBLK - 1:
                    cur_incl = incl_p.tile([1, D], F32R, tag="incl")
                    for c in range(2):
                        cs = slice(c * 512, (c + 1) * 512)
                        pst = tot_ps.tile([1, 512], FP32, tag="tot")
                        nc.tensor.matmul(
                            pst, lhsT=ones_col, rhs=xr_q[:, b, cs],
                            start=True, stop=True,
                        )
                        if i == 0:
                            nc.scalar.copy(out=cur_incl[0:1, cs], in_=pst)
                        else:
                            tot_sb = stat_p.tile([1, 512], FP32, tag="tot_sb")
                            nc.scalar.copy(out=tot_sb, in_=pst)
                            nc.gpsimd.tensor_add(
                                out=cur_incl[0:1, cs],
                                in0=prev_incl[0:1, cs],
                                in1=tot_sb,
                            )

                # -- in-block cumsum + carry, scaled to cumulative average ----
                for c in range(2):
                    cs = slice(c * 512, (c + 1) * 512)
                    ps = mm_ps.tile([P, 512], FP32, tag="mm")
                    nc.tensor.matmul(
                        ps, lhsT=tri_r, rhs=xr_q[:, b, cs],
                        start=True, stop=(i == 0),
                    )
                    if i > 0:
                        nc.tensor.matmul(
                            ps, lhsT=ones_row, rhs=prev_incl[0:1, cs],
                            start=False, stop=True,
                        )
                    nc.scalar.mul(out=avg_q[:, b, cs], in_=ps, mul=inv_sb[:, i : i + 1])
                if i < NBLK - 1:
                    prev_incl = cur_incl

                # -- LayerNorm stats + normalize (gain/bias folded into w1) ---
                st = stat_p.tile([P, 2, 6], FP32, tag="st")
                for g in range(2):
                    nc.vector.bn_stats(
                        out=st[:, g, :], in_=avg_q[:, b, g * 512 : (g + 1) * 512]
                    )
                mv = stat_p.tile([P, 2], FP32, tag="mv")
                nc.vector.bn_aggr(out=mv, in_=st)
                # rstd = 1/sqrt(var+eps) on DVE only: bit-hack seed + Newton
                y = stat_p.tile([P, 1], FP32, tag="y")
                nc.vector.tensor_scalar(
                    out=y, in0=mv[:, 1:2], scalar1=EPS, scalar2=None, op0=ALU.add
                )
                r0b = stat_p.tile([P, 1], mybir.dt.int32, tag="r0b")
                nc.vector.tensor_scalar(
                    out=r0b, in0=y[:].bitcast(mybir.dt.int32), scalar1=1,
                    scalar2=None, op0=ALU.logical_shift_right,
                )
                nc.vector.tensor_tensor(
                    out=r0b, in0=magic_sb, in1=r0b, op=ALU.subtract
                )
                rstd = r0b[:].bitcast(FP32)
                t = stat_p.tile([P, 1], FP32, tag="t")
                for _ in range(3):
                    nc.vector.tensor_tensor(out=t, in0=rstd, in1=rstd, op=ALU.mult)
                    nc.vector.tensor_tensor(out=t, in0=t, in1=y, op=ALU.mult)
                    nc.vector.tensor_scalar(
                        out=t, in0=t, scalar1=-0.5, scalar2=1.5,
                        op0=ALU.mult, op1=ALU.add,
                    )
                    nc.vector.tensor_tensor(out=rstd, in0=rstd, in1=t, op=ALU.mult)
                nc.vector.tensor_scalar(
                    out=z_q[:, b, :], in0=avg_q[:, b, :],
                    scalar1=mv[:, 0:1], scalar2=rstd,
                    op0=ALU.subtract, op1=ALU.mult,
                )
                if has_b2:
                    nc.gpsimd.tensor_add(
                        out=avg_q[:, b, :], in0=avg_q[:, b, :], in1=b2r_sb
                    )

                # -- transpose x block (bf16) for the gating matmul -----------
                xb = cast_p.tile([P, D], BF16, tag="xb")
                nc.vector.tensor_copy(out=xb, in_=x_q[:, b, :])
                transpose_blk(xb, xT, i * P)


            # -- transpose normalized activations: lnT [dpart, kc, cs] -------
            lnT_q = lnT_p.tile([P, KC, CS], FP8)
            for b in range(CB):
                transpose_blk(z_q[:, b, :], lnT_q, b * P)

            # -- FFN1: interT[f, s] = relu(w1'.T-chunks @ lnT + b1')/WS ------
            # (weights are host-scaled by WS; interT is stored /WS so FFN2's
            #  WS-scaled w2 cancels it — PSUM2 comes out unscaled.)
            intT_q = intT_p.tile([P, KC, CS], FP8)
            for fc in range(KC):
                ps = mm_ps.tile([P, 512], FP32, tag="mm")
                for k in range(0, KC, 2):
                    nc.tensor.matmul(
                        ps[:, :CS],
                        lhsT=w1_sb[:, k : k + 2, fc * P : (fc + 1) * P],
                        rhs=lnT_q[:, k : k + 2, :],
                        start=(k == 0), stop=(k == KC - 2),
                        perf_mode=DR,
                    )
                nc.scalar.activation(
                    out=intT_q[:, fc, :], in_=ps[:, :CS],
                    func=AF.Relu, bias=b1t_sb[:, fc : fc + 1],
                    scale=1.0 / (WS * WS),
                )

            # -- FFN2 + residual: avg_out = interT.T @ w2 + (avg + b2) -------
            ao_q = aoq_p.tile([P, CB, D], FP32)
            for b in range(CB):
                i = q * CB + b
                for dc in range(2):
                    ds_ = slice(dc * 512, (dc + 1) * 512)
                    ps = mm_ps.tile([P, 512], FP32, tag="mm")
                    for f in range(0, KC, 2):
                        nc.tensor.matmul(
                            ps,
                            lhsT=intT_q[:, f : f + 2, b * P : (b + 1) * P],
                            rhs=w2_sb[:, f : f + 2, ds_],
                            start=(f == 0), stop=(f == KC - 2),
                            perf_mode=DR,
                        )
                    nc.vector.tensor_add(
                        out=ao_q[:, b, ds_], in0=ps, in1=avg_q[:, b, ds_]
                    )
                nc.sync.dma_start(out=aout_r[:, i, :], in_=ao_q[:, b, :])
                aob = cast_p.tile([P, D], BF16, tag="aob")
                nc.vector.tensor_copy(out=aob, in_=ao_q[:, b, :])
                transpose_blk(aob, aoT, i * P)

        # -- gating ------------------------------------------------------------
        ctx.callback(lambda: None)
        ctx1.close()
        wg_p = ctx.enter_context(tc.tile_pool(name="wg", bufs=2))
        sig_p = ctx.enter_context(tc.tile_pool(name="sig", bufs=4))
        re_p = ctx.enter_context(tc.tile_pool(name="re", bufs=3))
        g_p = ctx.enter_context(tc.tile_pool(name="g", bufs=3))

        for dh in range(2):  # output feature half (512 wide)
            ds_ = slice(dh * 512, (dh + 1) * 512)
            wg_in = wg_p.tile([P, 2 * KC, 512], FP8, tag="wgin")
            wg_fg = wg_p.tile([P, 2 * KC, 512], FP8, tag="wgfg")
            for kh in range(2):
                ks = slice(kh * KC, (kh + 1) * KC)
                if not (dh == 0 and kh == 0):
                    nc.gpsimd.dma_start(
                        out=wg_in[:, ks, :], in_=wg_r[:, ks, dh * 512 : (dh + 1) * 512]
                    )
                nc.gpsimd.dma_start(
                    out=wg_fg[:, ks, :],
                    in_=wg_r[:, ks, D + dh * 512 : D + (dh + 1) * 512],
                )
            for sb in range(NBLK):
                scol = slice(sb * P, (sb + 1) * P)
                ps_pair = []
                for wi, wgt in enumerate((wg_in, wg_fg)):
                    ps = mm_ps.tile([P, 512], FP32, tag="mm")
                    for k in range(0, 2 * KC, 2):
                        lhs = (
                            xT[:, k : k + 2, scol]
                            if k < KC
                            else aoT[:, k - KC : k - KC + 2, scol]
                        )
                        if dh == 0 and wi == 0 and k < KC:
                            rhs = wg_pre[:, k : k + 2, :]
                        else:
                            rhs = wgt[:, k : k + 2, :]
                        nc.tensor.matmul(
                            ps, lhsT=lhs, rhs=rhs,
                            start=(k == 0), stop=(k == 2 * KC - 2),
                            perf_mode=DR,
                        )
                    ps_pair.append(ps)
                sig_in = sig_p.tile([P, 512], FP32, tag="sig")
                sig_fg = sig_p.tile([P, 512], FP32, tag="sig")
                if has_bg:
                    nc.vector.tensor_add(
                        out=sig_in, in0=ps_pair[0], in1=bgr_sb[:, ds_]
                    )
                    nc.scalar.activation(
                        out=sig_in, in_=sig_in, func=AF.Sigmoid, scale=1.0 / WS
                    )
                    nc.vector.tensor_add(
                        out=sig_fg, in0=ps_pair[1],
                        in1=bgr_sb[:, D + dh * 512 : D + (dh + 1) * 512],
                    )
                    nc.scalar.activation(
                        out=sig_fg, in_=sig_fg, func=AF.Sigmoid, scale=1.0 / WS
                    )
                else:
                    nc.scalar.activation(
                        out=sig_in, in_=ps_pair[0], func=AF.Sigmoid, scale=1.0 / WS
                    )
                    nc.scalar.activation(
                        out=sig_fg, in_=ps_pair[1], func=AF.Sigmoid, scale=1.0 / WS
                    )

                x_re = re_p.tile([P, 512], F32R, tag="xre")
                nc.sync.dma_start(out=x_re, in_=x_d[sb * P : (sb + 1) * P, ds_])
                ao_re = re_p.tile([P, 512], FP32, tag="aore")
                nc.sync.dma_start(out=ao_re, in_=aout_d[sb * P : (sb + 1) * P, ds_])
                m1 = g_p.tile([P, 512], FP32, tag="m1")
                nc.vector.tensor_mul(out=m1, in0=sig_in, in1=x_re)
                m2 = g_p.tile([P, 512], FP32, tag="m2")
                nc.gpsimd.tensor_mul(out=m2, in0=sig_fg, in1=ao_re)
                gt = g_p.tile([P, 512], FP32, tag="gt")
                nc.vector.tensor_add(out=gt, in0=m1, in1=m2)
                nc.sync.dma_start(out=gated_d[sb * P : (sb + 1) * P, ds_], in_=gt)

    nc.compile()
    return nc


def host_inputs(x, w1, b1, w2, b2, ln_g, ln_b, wg, bg):
    """Fold LN affine params into w1/b1, precompute constants, cast weights."""
    x = np.asarray(x, np.float32)
    w1 = np.asarray(w1, np.float32)
    w2 = np.asarray(w2, np.float32)
    wg = np.asarray(wg, np.float32)
    ln_g = np.asarray(ln_g, np.float32)
    ln_b = np.asarray(ln_b, np.float32)
    b1 = np.asarray(b1, np.float32)

    WS = 16.0
    w1g = (ln_g[:, None] * w1 * WS).astype(ml_dtypes.float8_e4m3)
    b1p = ((b1 + ln_b @ w1) / WS).astype(np.float32)
    tri = np.triu(np.ones((P, P), np.float32))
    iden = np.eye(P, dtype=ml_dtypes.bfloat16)
    pos = np.arange(S, dtype=np.float64).reshape(NBLK, P).T  # [P, NBLK]
    invpos = (1.0 / (pos + 1.0)).astype(np.float32)

    base = {
        "x": None,  # per-core
        "w1g": w1g,
        "b1p": b1p,
        "w2": (w2 * WS).astype(ml_dtypes.float8_e4m3),
        "wg": (wg * WS).astype(ml_dtypes.float8_e4m3),
        "tri": tri,
        "iden": iden,
        "invpos": invpos,
    }
    has_b2 = bool(np.any(b2))
    has_bg = bool(np.any(bg))
    if has_b2:
        base["b2"] = np.asarray(b2, np.float32)
    if has_bg:
        base["bg"] = (np.asarray(bg, np.float32) * WS).astype(np.float32)
    return base, has_b2, has_bg


_prog_cache = {}


def kernel(x, w1, b1, w2, b2, ln_g, ln_b, wg, bg):
    x = np.asarray(x, np.float32)
    assert x.shape == (B, S, D), x.shape
    base, has_b2, has_bg = host_inputs(x, w1, b1, w2, b2, ln_g, ln_b, wg, bg)

    key = (has_b2, has_bg)
    if key not in _prog_cache:
        _prog_cache[key] = build_program(has_b2, has_bg)
    nc = _prog_cache[key]

    in_maps = []
    for core in range(B):
        m = dict(base)
        m["x"] = np.ascontiguousarray(x[core])
        in_maps.append(m)

    res = run_bass_kernel_spmd(nc, in_maps, core_ids=list(range(B)))
    gated = np.stack([res.results[c]["gated"] for c in range(B)])
    avg_out = np.stack([res.results[c]["avg_out"] for c in range(B)])
    return gated, avg_out



# revision 26
# speedup vs baseline: 1.7816x; 1.0020x over previous
"""AverageAttention Trainium2 kernel.

Computes, per batch b (data-parallel across 8 NeuronCores):
    avg      = cumsum(x, axis=seq) / (pos+1)
    inter    = relu(LN(avg) @ w1 + b1)
    avg_out  = inter @ w2 + b2 + avg
    gates    = [x, avg_out] @ wg + bg
    gated    = sigmoid(gates[:, :D]) * x + sigmoid(gates[:, D:]) * avg_out
returns (gated, avg_out), each [B, S, D].

Implementation notes:
  - cumsum via triangular matmul per 128-seq block (fp32r streaming) + a K=1
    rank-1 matmul adding the running carry into PSUM, scaled by 1/(pos+1) at
    eviction (per-partition scale on ScalarE). The serial carry chain is a
    single fused DVE tensor_tensor per half (prev + tot-PSUM).
  - LayerNorm gain/bias are folded into w1/b1 on the host; on-chip LN is just
    (x-mu)*rstd via bn_stats/bn_aggr + one tensor_scalar (rstd via DVE-only
    fast-inverse-sqrt so no extra ACT table set is needed).
  - FFN and gating matmuls run in fp8e4 (e4m3) with DoubleRow perf mode
    (2 K-chunks per instruction). Weights are host-scaled by WS=16 to avoid
    fp8 subnormals; descaled at PSUM eviction via ACT scale.
  - Activations are transposed on the PE straight from fp32r (no cast) with a
    bf16 identity; PSUM transposes evict to fp8 SBUF on ScalarE.
  - x stays resident in SBUF for the whole kernel (no gating-phase re-read);
    avg_out is re-read from DRAM for the final elementwise gating.
"""

import os
import sys

if "/opt/trn_rl_repo" not in sys.path:
    sys.path.insert(0, "/opt/trn_rl_repo")

# The NEFF executes via the axon-tunneled PJRT backend; a JAX_PLATFORMS=cpu
# pin (used for running references) would hide the NeuronCores.
if os.environ.get("JAX_PLATFORMS") == "cpu":
    os.environ.pop("JAX_PLATFORMS")

from contextlib import ExitStack

import ml_dtypes
import numpy as np

import concourse.bass as bass
import concourse.mybir as mybir
import concourse.tile as tile
from concourse import bacc
from concourse.bass_utils import run_bass_kernel_spmd

B, S, D = 8, 2048, 1024
P = 128
NBLK = S // P            # 16 seq blocks per core
CB = 2                   # seq blocks per pipeline chunk
NCHUNK = NBLK // CB
CS = CB * P              # chunk seq length (256)
D2 = 2 * D
KC = D // P              # 8 feature chunks of 128
EPS = 1e-6

FP32 = mybir.dt.float32
BF16 = mybir.dt.bfloat16
F32R = mybir.dt.float32r
FP8 = mybir.dt.float8e4
WS = 16.0                     # host-side weight scale (keeps fp8 normal-range)

AF = mybir.ActivationFunctionType
ALU = mybir.AluOpType
DR = mybir.MatmulPerfMode.DoubleRow


def build_program(has_b2: bool, has_bg: bool) -> bacc.Bacc:
    nc = bacc.Bacc("TRN2", target_bir_lowering=False, debug=False, num_devices=8)

    x_d = nc.declare_dram_parameter("x", [S, D], F32R, isOutput=False)
    w1_d = nc.declare_dram_parameter("w1g", [D, D], FP8, isOutput=False)
    b1_d = nc.declare_dram_parameter("b1p", [D], FP32, isOutput=False)
    w2_d = nc.declare_dram_parameter("w2", [D, D], FP8, isOutput=False)
    wg_d = nc.declare_dram_parameter("wg", [D2, D2], FP8, isOutput=False)
    tri_d = nc.declare_dram_parameter("tri", [P, P], F32R, isOutput=False)
    iden_d = nc.declare_dram_parameter("iden", [P, P], F32R, isOutput=False)
    inv_d = nc.declare_dram_parameter("invpos", [P, NBLK], FP32, isOutput=False)
    if has_b2:
        b2_d = nc.declare_dram_parameter("b2", [D], FP32, isOutput=False)
    if has_bg:
        bg_d = nc.declare_dram_parameter("bg", [D2], FP32, isOutput=False)

    gated_d = nc.declare_dram_parameter("gated", [S, D], FP32, isOutput=True)
    aout_d = nc.declare_dram_parameter("avg_out", [S, D], FP32, isOutput=True)

    x_r = x_d[:].rearrange("(n p) d -> p n d", p=P)        # [128, 16, 1024]
    aout_r = aout_d[:].rearrange("(n p) d -> p n d", p=P)
    gated_r = gated_d[:].rearrange("(n p) d -> p n d", p=P)
    w1_r = w1_d[:].rearrange("(c p) f -> p c f", p=P)      # [128, 8, 1024]
    w2_r = w2_d[:].rearrange("(c p) f -> p c f", p=P)
    wg_r = wg_d[:].rearrange("(c p) j -> p c j", p=P)      # [128, 16, 2048]

    with tile.TileContext(nc) as tc, ExitStack() as ctx:
        const = ctx.enter_context(tc.tile_pool(name="const", bufs=1))

        xT = const.tile([P, KC, S], FP8)       # x transposed, for gating lhsT
        aoT = const.tile([P, KC, S], FP8)      # avg_out transposed
        xs = const.tile([P, NBLK, D], F32R)    # x resident (fp32) for gating

        mm_ps = ctx.enter_context(tc.tile_pool(name="mm_ps", bufs=4, space="PSUM"))
        tot_ps = ctx.enter_context(tc.tile_pool(name="tot_ps", bufs=1, space="PSUM"))
        tr_ps = ctx.enter_context(tc.tile_pool(name="tr_ps", bufs=2, space="PSUM"))

        ctx1 = ctx.enter_context(ExitStack())
        w12 = ctx1.enter_context(tc.tile_pool(name="w12", bufs=1))
        avgq_p = ctx1.enter_context(tc.tile_pool(name="avgq", bufs=2))
        zq_p = ctx1.enter_context(tc.tile_pool(name="zq", bufs=2))
        lnT_p = ctx1.enter_context(tc.tile_pool(name="lnT", bufs=2))
        intT_p = ctx1.enter_context(tc.tile_pool(name="intT", bufs=2))
        aoq_p = ctx1.enter_context(tc.tile_pool(name="aoq", bufs=2))
        stat_p = ctx1.enter_context(tc.tile_pool(name="stat", bufs=6))
        incl_p = ctx1.enter_context(tc.tile_pool(name="incl", bufs=2))

        def transpose_blk(src_ap, src_dt, dst_tile, dst_scol):
            """Transpose a [128, 1024] fp32-width block into
            dst_tile[:, :, dst_scol:+128] as fp8.

            8 PE transposes batched 4-per-PSUM-bank, evicted (and cast to the
            dst dtype) on ScalarE. DMA-rounded f32r sources stream at 1.5
            cyc/row; compute-produced fp32 sources at 2 cyc/row."""
            iden = iden_sb if src_dt is F32R else idenf_sb
            for h in range(2):
                ptr = tr_ps.tile([P, 512], src_dt, tag="tr")
                for j in range(4):
                    k = 4 * h + j
                    nc.tensor.transpose(
                        ptr[:, j * P : (j + 1) * P],
                        src_ap[:, k * P : (k + 1) * P],
                        iden,
                    )
                ev = ptr[:].rearrange("p (j s) -> p j s", j=4)
                if src_dt is F32R:
                    ev = ev.bitcast(FP32)
                nc.scalar.copy(
                    out=dst_tile[:, 4 * h : 4 * h + 4, dst_scol : dst_scol + P],
                    in_=ev,
                )

        def issue_x(i0):
            for i in range(i0, min(i0 + CB, NBLK)):
                nc.sync.dma_start(out=xs[:, i, :], in_=x_r[:, i, :])

        issue_x(0)
        issue_x(CB)

        iden_sb = const.tile([P, P], F32R)
        nc.sync.dma_start(out=iden_sb, in_=iden_d[:])
        idenf_sb = const.tile([P, P], FP32)
        nc.sync.dma_start(out=idenf_sb, in_=iden_d[:].bitcast(FP32))
        inv_sb = const.tile([P, NBLK], FP32)
        nc.sync.dma_start(out=inv_sb, in_=inv_d[:])
        b1t_sb = const.tile([P, KC], FP32)
        nc.sync.dma_start(out=b1t_sb, in_=b1_d[:].rearrange("(c p) -> p c", p=P))
        # int32 seed constant for the DVE fast-inverse-sqrt (keeps Sqrt off
        # ScalarE so the whole kernel fits one ACT table set — no mid-kernel
        # LoadActFuncSet switch before the gating sigmoids)
        magic_sb = const.tile([P, 1], mybir.dt.int32)
        nc.vector.memset(magic_sb, 0x5F3759DF)
        if has_b2:
            b2r_sb = const.tile([P, D], FP32)
            nc.sync.dma_start(out=b2r_sb, in_=b2_d[None, :].to_broadcast([P, D]))
        if has_bg:
            bgr_sb = const.tile([P, D2], FP32)
            nc.sync.dma_start(out=bgr_sb, in_=bg_d[None, :].to_broadcast([P, D2]))

        # fp32r operands may be DMA'd directly when the buffer dtype is f32r
        tri_rsb = const.tile([P, P], F32R)
        nc.sync.dma_start(out=tri_rsb, in_=tri_d[:])
        tri_r = tri_rsb[:]
        ones_row = tri_rsb[0:1, :]             # row of ones [1, 128]
        ones_col = tri_rsb[:, P - 1 : P]       # column of ones [128, 1]

        wg_pre = const.tile([P, KC, 512], FP8)  # wg[:, k<8, j 0:512] prefetch

        w1_sb = w12.tile([P, KC, D], FP8)
        nc.sync.dma_start(out=w1_sb, in_=w1_r)
        w2_sb = w12.tile([P, KC, D], FP8)
        nc.sync.dma_start(out=w2_sb, in_=w2_r)

        prev_incl = None
        for q in range(NCHUNK):
            issue_x((q + 2) * CB)
            if q == NCHUNK - 2:
                nc.gpsimd.dma_start(out=wg_pre, in_=wg_r[:, :KC, 0:512])
            avg_q = avgq_p.tile([P, CB, D], FP32)
            z_q = zq_p.tile([P, CB, D], FP32)

            for b in range(CB):
                i = q * CB + b
                # -- block totals + running prefix (serial carry on DVE) ------
                if i < NBLK - 1:
                    cur_incl = incl_p.tile([1, D], F32R, tag="incl")
                    for c in range(2):
                        cs = slice(c * 512, (c + 1) * 512)
                        pst = tot_ps.tile([1, 512], FP32, tag="tot")
                        nc.tensor.matmul(
                            pst, lhsT=ones_col, rhs=xs[:, i, cs],
                            start=True, stop=True,
                        )
                        if i == 0:
                            nc.vector.tensor_copy(out=cur_incl[0:1, cs], in_=pst)
                        else:
                            nc.vector.tensor_tensor(
                                out=cur_incl[0:1, cs],
                                in0=prev_incl[0:1, cs],
                                in1=pst,
                                op=ALU.add,
                            )

                # -- in-block cumsum + carry, scaled to cumulative average ----
                for c in range(2):
                    cs = slice(c * 512, (c + 1) * 512)
                    ps = mm_ps.tile([P, 512], FP32, tag="mm")
                    nc.tensor.matmul(
                        ps, lhsT=tri_r, rhs=xs[:, i, cs],
                        start=True, stop=(i == 0),
                    )
                    if i > 0:
                        nc.tensor.matmul(
                            ps, lhsT=ones_row, rhs=prev_incl[0:1, cs],
                            start=False, stop=True,
                        )
                    nc.scalar.mul(out=avg_q[:, b, cs], in_=ps, mul=inv_sb[:, i : i + 1])
                if i < NBLK - 1:
                    prev_incl = cur_incl

                # -- LayerNorm stats + normalize (gain/bias folded into w1) ---
                st = stat_p.tile([P, 2, 6], FP32, tag="st")
                for g in range(2):
                    nc.vector.bn_stats(
                        out=st[:, g, :], in_=avg_q[:, b, g * 512 : (g + 1) * 512]
                    )
                mv = stat_p.tile([P, 2], FP32, tag="mv")
                nc.vector.bn_aggr(out=mv, in_=st)
                # rstd = 1/sqrt(var+eps) on DVE only: bit-hack seed + Newton
                y = stat_p.tile([P, 1], FP32, tag="y")
                nc.vector.tensor_scalar(
                    out=y, in0=mv[:, 1:2], scalar1=EPS, scalar2=None, op0=ALU.add
                )
                r0b = stat_p.tile([P, 1], mybir.dt.int32, tag="r0b")
                nc.vector.tensor_scalar(
                    out=r0b, in0=y[:].bitcast(mybir.dt.int32), scalar1=1,
                    scalar2=None, op0=ALU.logical_shift_right,
                )
                nc.vector.tensor_tensor(
                    out=r0b, in0=magic_sb, in1=r0b, op=ALU.subtract
                )
                rstd = r0b[:].bitcast(FP32)
                t = stat_p.tile([P, 1], FP32, tag="t")
                for _ in range(3):
                    nc.vector.tensor_tensor(out=t, in0=rstd, in1=rstd, op=ALU.mult)
                    nc.vector.tensor_tensor(out=t, in0=t, in1=y, op=ALU.mult)
                    nc.vector.tensor_scalar(
                        out=t, in0=t, scalar1=-0.5, scalar2=1.5,
                        op0=ALU.mult, op1=ALU.add,
                    )
                    nc.vector.tensor_tensor(out=rstd, in0=rstd, in1=t, op=ALU.mult)
                nc.vector.tensor_scalar(
                    out=z_q[:, b, :], in0=avg_q[:, b, :],
                    scalar1=mv[:, 0:1], scalar2=rstd,
                    op0=ALU.subtract, op1=ALU.mult,
                )
                if has_b2:
                    nc.gpsimd.tensor_add(
                        out=avg_q[:, b, :], in0=avg_q[:, b, :], in1=b2r_sb
                    )

                # -- transpose x block (fp32r, no cast) for the gating matmul -
                transpose_blk(xs[:, i, :], F32R, xT, i * P)

            # -- transpose normalized activations: lnT [dpart, kc, cs] -------
            lnT_q = lnT_p.tile([P, KC, CS], FP8)
            for b in range(CB):
                transpose_blk(z_q[:, b, :], FP32, lnT_q, b * P)

            # -- FFN1: interT[f, s] = relu(w1'.T-chunks @ lnT + b1')/WS ------
            # (weights are host-scaled by WS; interT is stored /WS so FFN2's
            #  WS-scaled w2 cancels it — PSUM2 comes out unscaled.)
            intT_q = intT_p.tile([P, KC, CS], FP8)
            for fc in range(KC):
                ps = mm_ps.tile([P, 512], FP32, tag="mm")
                for k in range(0, KC, 2):
                    nc.tensor.matmul(
                        ps[:, :CS],
                        lhsT=w1_sb[:, k : k + 2, fc * P : (fc + 1) * P],
                        rhs=lnT_q[:, k : k + 2, :],
                        start=(k == 0), stop=(k == KC - 2),
                        perf_mode=DR,
                    )
                nc.scalar.activation(
                    out=intT_q[:, fc, :], in_=ps[:, :CS],
                    func=AF.Relu, bias=b1t_sb[:, fc : fc + 1],
                    scale=1.0 / (WS * WS),
                )

            # -- FFN2 + residual: avg_out = interT.T @ w2 + (avg + b2) -------
            ao_q = aoq_p.tile([P, CB, D], FP32)
            for b in range(CB):
                i = q * CB + b
                for dc in range(2):
                    ds_ = slice(dc * 512, (dc + 1) * 512)
                    ps = mm_ps.tile([P, 512], FP32, tag="mm")
                    for f in range(0, KC, 2):
                        nc.tensor.matmul(
                            ps,
                            lhsT=intT_q[:, f : f + 2, b * P : (b + 1) * P],
                            rhs=w2_sb[:, f : f + 2, ds_],
                            start=(f == 0), stop=(f == KC - 2),
                            perf_mode=DR,
                        )
                    nc.vector.tensor_add(
                        out=ao_q[:, b, ds_], in0=ps, in1=avg_q[:, b, ds_]
                    )
                nc.sync.dma_start(out=aout_r[:, i, :], in_=ao_q[:, b, :])
                transpose_blk(ao_q[:, b, :], FP32, aoT, i * P)

        # -- gating ------------------------------------------------------------
        ctx.callback(lambda: None)
        ctx1.close()
        wg_p = ctx.enter_context(tc.tile_pool(name="wg", bufs=1))
        sig_p = ctx.enter_context(tc.tile_pool(name="sig", bufs=4))
        re_p = ctx.enter_context(tc.tile_pool(name="re", bufs=3))
        g_p = ctx.enter_context(tc.tile_pool(name="g", bufs=3))

        for dh in range(2):  # output feature half (512 wide)
            ds_ = slice(dh * 512, (dh + 1) * 512)
            wg_in = wg_p.tile([P, 2 * KC, 512], FP8, tag="wgin")
            wg_fg = wg_p.tile([P, 2 * KC, 512], FP8, tag="wgfg")
            for kh in range(2):
                ks = slice(kh * KC, (kh + 1) * KC)
                if not (dh == 0 and kh == 0):
                    nc.gpsimd.dma_start(
                        out=wg_in[:, ks, :], in_=wg_r[:, ks, dh * 512 : (dh + 1) * 512]
                    )
                nc.gpsimd.dma_start(
                    out=wg_fg[:, ks, :],
                    in_=wg_r[:, ks, D + dh * 512 : D + (dh + 1) * 512],
                )
            for sb in range(NBLK):
                scol = slice(sb * P, (sb + 1) * P)
                ps_pair = []
                for wi, wgt in enumerate((wg_in, wg_fg)):
                    ps = mm_ps.tile([P, 512], FP32, tag="mm")
                    for k in range(0, 2 * KC, 2):
                        lhs = (
                            xT[:, k : k + 2, scol]
                            if k < KC
                            else aoT[:, k - KC : k - KC + 2, scol]
                        )
                        if dh == 0 and wi == 0 and k < KC:
                            rhs = wg_pre[:, k : k + 2, :]
                        else:
                            rhs = wgt[:, k : k + 2, :]
                        nc.tensor.matmul(
                            ps, lhsT=lhs, rhs=rhs,
                            start=(k == 0), stop=(k == 2 * KC - 2),
                            perf_mode=DR,
                        )
                    ps_pair.append(ps)
                sig_in = sig_p.tile([P, 512], FP32, tag="sig")
                sig_fg = sig_p.tile([P, 512], FP32, tag="sig")
                if has_bg:
                    nc.vector.tensor_add(
                        out=sig_in, in0=ps_pair[0], in1=bgr_sb[:, ds_]
                    )
                    nc.scalar.activation(
                        out=sig_in, in_=sig_in, func=AF.Sigmoid, scale=1.0 / WS
                    )
                    nc.vector.tensor_add(
                        out=sig_fg, in0=ps_pair[1],
                        in1=bgr_sb[:, D + dh * 512 : D + (dh + 1) * 512],
                    )
                    nc.scalar.activation(
                        out=sig_fg, in_=sig_fg, func=AF.Sigmoid, scale=1.0 / WS
                    )
                else:
                    nc.scalar.activation(
                        out=sig_in, in_=ps_pair[0], func=AF.Sigmoid, scale=1.0 / WS
                    )
                    nc.scalar.activation(
                        out=sig_fg, in_=ps_pair[1], func=AF.Sigmoid, scale=1.0 / WS
                    )

                ao_re = re_p.tile([P, 512], FP32, tag="aore")
                nc.sync.dma_start(out=ao_re, in_=aout_d[sb * P : (sb + 1) * P, ds_])
                m1 = g_p.tile([P, 512], FP32, tag="m1")
                nc.gpsimd.tensor_mul(out=m1, in0=sig_in, in1=xs[:, sb, ds_])
                m2 = g_p.tile([P, 512], FP32, tag="m2")
                nc.gpsimd.tensor_mul(out=m2, in0=sig_fg, in1=ao_re)
                gt = g_p.tile([P, 512], FP32, tag="gt")
                nc.vector.tensor_add(out=gt, in0=m1, in1=m2)
                nc.sync.dma_start(out=gated_d[sb * P : (sb + 1) * P, ds_], in_=gt)

    nc.compile()
    return nc


def host_inputs(x, w1, b1, w2, b2, ln_g, ln_b, wg, bg):
    """Fold LN affine params into w1/b1, precompute constants, cast weights."""
    x = np.asarray(x, np.float32)
    w1 = np.asarray(w1, np.float32)
    w2 = np.asarray(w2, np.float32)
    wg = np.asarray(wg, np.float32)
    ln_g = np.asarray(ln_g, np.float32)
    ln_b = np.asarray(ln_b, np.float32)
    b1 = np.asarray(b1, np.float32)

    w1g = (ln_g[:, None] * w1 * WS).astype(ml_dtypes.float8_e4m3)
    b1p = ((b1 + ln_b @ w1) / WS).astype(np.float32)
    tri = np.triu(np.ones((P, P), np.float32))
    iden = np.eye(P, dtype=np.float32)  # loaded as both f32r and fp32
    pos = np.arange(S, dtype=np.float64).reshape(NBLK, P).T  # [P, NBLK]
    invpos = (1.0 / (pos + 1.0)).astype(np.float32)

    base = {
        "x": None,  # per-core
        "w1g": w1g,
        "b1p": b1p,
        "w2": (w2 * WS).astype(ml_dtypes.float8_e4m3),
        "wg": (wg * WS).astype(ml_dtypes.float8_e4m3),
        "tri": tri,
        "iden": iden,
        "invpos": invpos,
    }
    has_b2 = bool(np.any(b2))
    has_bg = bool(np.any(bg))
    if has_b2:
        base["b2"] = np.asarray(b2, np.float32)
    if has_bg:
        base["bg"] = (np.asarray(bg, np.float32) * WS).astype(np.float32)
    return base, has_b2, has_bg


_prog_cache = {}


def kernel(x, w1, b1, w2, b2, ln_g, ln_b, wg, bg):
    x = np.asarray(x, np.float32)
    assert x.shape == (B, S, D), x.shape
    base, has_b2, has_bg = host_inputs(x, w1, b1, w2, b2, ln_g, ln_b, wg, bg)

    key = (has_b2, has_bg)
    if key not in _prog_cache:
        _prog_cache[key] = build_program(has_b2, has_bg)
    nc = _prog_cache[key]

    in_maps = []
    for core in range(B):
        m = dict(base)
        m["x"] = np.ascontiguousarray(x[core])
        in_maps.append(m)

    res = run_bass_kernel_spmd(nc, in_maps, core_ids=list(range(B)))
    gated = np.stack([res.results[c]["gated"] for c in range(B)])
    avg_out = np.stack([res.results[c]["avg_out"] for c in range(B)])
    return gated, avg_out
